# revision 5
# baseline (speedup 1.0000x reference)
"""Bezier-to-image Gaussian splat kernel for Trainium2 (8 NeuronCores).

Reference computation (per sample b of 256):
    T = warped cubic Bernstein basis (30, 4)
    points = einsum('nk,blkc->blnc', T, x.reshape(B,160,4,2))   # (B,160,30,2)
    gx[b,l,i,n] = exp(-(i/60 - X[b,l,n])^2 / 2e-4)
    out[b,i,j]  = min(sum_{l,n} gx[b,l,i,n]*gy[b,l,j,n], 1)     # (B,60,60)

Strategy: pure data parallel, 32 samples per core.  The host pre-transposes
control points into a [20, 2560] layout (4 curve-strips x (4 ctrl rows +
ones row)) so the whole input is ONE contiguous DMA, and a single
block-diagonal [20,128] stationary computes r256 = round(256*60*X) for a
PAIR of samples per matmul.  The banded distance d256 = 256*i - r256 is an
all-int16 packed tensor_tensor (DVE 2x mode); the Gaussian is ONE
Derivative_Erf activation per pair (cost on ACT is free-size only), whose
output AP permutes the band into chunk-blocked layout for the 60x60 PSUM
accumulation matmuls.  Emission is software-pipelined (r two pairs ahead,
subtracts one pair ahead, clamps one pair behind) to keep ACT ~100% busy:
ACT is the roofline engine at ~8.4us/pair.
"""

import math

import numpy as np
import orjson

import bass_rust
import concourse.bass as bass
import concourse.mybir as mybir
import concourse.tile as tile
from concourse.bass_utils import run_bass_kernel_spmd

B, L, N, W = 256, 160, 30, 60
NCORES = 8
BC = B // NCORES          # samples per core (32)
NPAIR = BC // 2           # 16
ALPHA = 2e-4
KEXP = 1.0 / (W * W * ALPHA)          # exponent scale in cell units: 1/0.72
SDERF = math.sqrt(KEXP)               # Derivative_Erf input scale (per cell)
DERF_FIX = math.pi / 4.0              # undo (2/sqrt(pi))^2 from Derivative_Erf
CHUNKS = 40                           # 4 curves x 30 samples per chunk
PTS = 128                             # chunk partition dim: p = 32*lg + n
CW = 60                               # width of one chunk's band (= W)
R_HOLE = -15360.0                     # r256 for dead rows -> d256 large -> g=0
Q = 256.0                             # fixed-point scale (1/256 cell)

# If the hardware rejects int16 activation input, set to True: inserts a
# 4x-mode tensor_scalar converting d256 -> f16 cells before the LUT.
ACT_F16_FALLBACK = False

LAST_RESULTS = None  # test harness reads profiling info from here


def _basis_T() -> np.ndarray:
    t = np.arange(N, dtype=np.float32) / np.float32(N)
    t = 2 * t**3 - 3 * t**2 + 2 * t
    t_3_0 = t**3
    t_2_1 = t**2 - t_3_0
    t_1_2 = t_3_0 - 2 * t**2 + t
    t_0_3 = (1 - t) ** 3
    return np.stack([t_3_0, 3 * t_2_1, 3 * t_1_2, t_0_3], axis=1).astype(np.float32)


def _legalize_waits(nc, max_waits: int = 1):
    """Walrus rejects engine instructions carrying more than ~1 sync wait
    ("Too many sync wait commands").  Hoist excess waits onto same-engine
    Drain instructions inserted immediately before the offender."""
    js = orjson.loads(mybir.module_to_json_bytes(nc.m))
    ctr = 0
    for f in js["functions"]:
        for bb in f["blocks"]:
            out = []
            changed = False
            for inst in bb["instructions"]:
                si = inst.get("sync_info")
                waits = si.get("on_wait") if si else None
                if waits and len(waits) > max_waits:
                    keep = waits[:max_waits]
                    for w in waits[max_waits:]:
                        ctr += 1
                        out.append({
                            "debug": inst.get("debug", 0),
                            "engine": inst["engine"],
                            "ins": [], "outs": [],
                            "name": f"waitfix-{ctr}",
                            "opcode": "Drain",
                            "sync_info": {"on_update": [], "on_wait": [w]},
                        })
                    si["on_wait"] = keep
                    changed = True
                out.append(inst)
            if changed:
                bb["instructions"] = out
    if ctr:
        nc.m = bass_rust.module_from_json_bytes(orjson.dumps(js))
    return ctr


def _host_ctrl(x_core: np.ndarray) -> np.ndarray:
    """[32,160,8] f32 -> [20, 2560] f32: row 5*lg+k col (b,c,cc) =
    x[b, 4c+lg, 2k+cc] for k<4; row 5*lg+4 = 1.0 (ones row for holes)."""
    xr = x_core.reshape(BC, CHUNKS, 4, 4, 2)          # b, c, lg, k, cc
    arr = np.ones((4, 5, BC, CHUNKS, 2), dtype=np.float32)
    arr[:, :4] = xr.transpose(2, 3, 0, 1, 4)          # lg, k, b, c, cc
    return np.ascontiguousarray(arr.reshape(20, BC * CHUNKS * 2))


def build_program(legalize: bool = True):
    f32 = mybir.dt.float32
    f16 = mybir.dt.float16
    i16 = mybir.dt.int16

    nc = bass.Bass("TRN2", target_bir_lowering=False, debug=False)

    x_t = nc.dram_tensor("x", [20, BC * CHUNKS * 2], f32, kind="ExternalInput")
    y_t = nc.dram_tensor("y", [BC, W, W], f32, kind="ExternalOutput")

    # Block-diagonal stationary: col m = 32*lg + n gets 256*60*T[n,k] from
    # row 5*lg+k; hole cols n in {30,31} get R_HOLE via the ones row 5*lg+4.
    tsc_np = np.zeros((20, 128), dtype=np.float32)
    Tb = (Q * W) * _basis_T()                         # (30, 4)
    for lg in range(4):
        tsc_np[5 * lg : 5 * lg + 4, 32 * lg : 32 * lg + 30] = Tb.T
        tsc_np[5 * lg + 4, 32 * lg + 30 : 32 * lg + 32] = R_HOLE
    tsc_d = nc.inline_tensor(tsc_np, name="tscT")

    # iota: value 256*w at offset (w, s): [128, 120] int16
    iota_np = np.tile(
        np.repeat((Q * np.arange(CW)).astype(np.int16), 2)[None, :], (PTS, 1)
    )
    iota_d = nc.inline_tensor(iota_np, name="iota256")

    PAIR_F = 2 * CHUNKS * CW * 2                      # 9600 band elems per pair
    SAMP_F = 2 * CHUNKS * CW                          # 4800 per sample
    NSLICE = 8                                        # input DMA slices
    SCOL = (BC * CHUNKS * 2) // NSLICE                # 320 cols per slice

    with tile.TileContext(nc) as tc, tc.tile_pool(name="const", bufs=1) as cpool, \
            tc.tile_pool(name="ctrl", bufs=1) as ctrl_pool, \
            tc.tile_pool(name="outp", bufs=2) as out_pool, \
            tc.tile_pool(name="dd", bufs=2) as dd_pool, \
            tc.tile_pool(name="gg", bufs=2) as gg_pool, \
            tc.tile_pool(name="rps", bufs=2, space="PSUM") as rps_pool, \
            tc.tile_pool(name="img", bufs=4, space="PSUM") as img_pool:

        tsc = cpool.tile([20, 128], f32, tag="tsc")
        nc.sync.dma_start(tsc[:], tsc_d.ap())
        iot = cpool.tile([PTS, 2 * CW], i16, tag="iota")
        nc.sync.dma_start(iot[:], iota_d.ap())

        # control points: 8 independent column-slice tiles so each pair's
        # matmul only waits on its own slice's DMA.
        cts = []
        for s in range(NSLICE):
            ct_s = ctrl_pool.tile([20, SCOL], f32, tag=f"ct{s}")
            nc.sync.dma_start(ct_s[:], x_t.ap()[:, s * SCOL : (s + 1) * SCOL])
            cts.append(ct_s)

        # r256 for all 16 pairs lives in one persistent tile
        r_all = ctrl_pool.tile([PTS, NPAIR * 160], i16, tag="rall")

        dd_t = [None] * NPAIR
        gg_t = [None] * NPAIR
        img_t = [None] * BC
        outp_t = [None] * NPAIR

        def emit_rmm(P):
            sl = cts[P // 2]
            off = (P % 2) * 160
            r_ps = rps_pool.tile([PTS, 160], f32, tag="rps")
            nc.tensor.matmul(
                r_ps[:], lhsT=tsc[:], rhs=sl[:, off : off + 160],
                start=True, stop=True,
            )
            nc.vector.tensor_copy(r_all[:, 160 * P : 160 * P + 160], r_ps[:])

        def emit_sub(P):
            dd = dd_pool.tile([PTS, PAIR_F], i16, tag="dd")
            dd_t[P] = dd
            for b2 in range(2):
                # d256[p, (w, c, s)] = 256*w - r256[p, (c, s)]
                nc.vector.tensor_tensor(
                    dd[:, SAMP_F * b2 : SAMP_F * (b2 + 1)].rearrange(
                        "p (w c s) -> p w c s", w=CW, c=CHUNKS, s=2
                    ),
                    iot[:].rearrange("p (w o s) -> p w o s", o=1, s=2)
                    .broadcast_to([PTS, CW, CHUNKS, 2]),
                    r_all[:, 160 * P + 80 * b2 : 160 * P + 80 * (b2 + 1)]
                    .rearrange("p (o c s) -> p o c s", o=1, s=2)
                    .broadcast_to([PTS, CW, CHUNKS, 2]),
                    mybir.AluOpType.subtract,
                )

        def emit_act(P):
            gg = gg_pool.tile([PTS, PAIR_F], f16, tag="gg")
            gg_t[P] = gg
            dd = dd_t[P]
            # out written chunk-blocked: (b, w, cs) iteration -> (b, cs, w)
            gg_v = gg[:].rearrange("p (b cs w) -> p b w cs", b=2, cs=2 * CHUNKS, w=CW)
            if ACT_F16_FALLBACK:
                df = dd_pool.tile([PTS, PAIR_F], f16, tag="df")
                nc.vector.tensor_scalar(
                    df[:], dd[:], 1.0 / Q, 0.0,
                    mybir.AluOpType.mult, mybir.AluOpType.add,
                )
                src = df[:].rearrange("p (b w cs) -> p b w cs", b=2, w=CW)
                nc.scalar.activation(
                    gg_v, src, mybir.ActivationFunctionType.Derivative_Erf,
                    bias=0.0, scale=SDERF,
                )
            else:
                src = dd[:].rearrange("p (b w cs) -> p b w cs", b=2, w=CW)
                nc.scalar.activation(
                    gg_v, src, mybir.ActivationFunctionType.Derivative_Erf,
                    bias=0.0, scale=SDERF / Q,
                )

        def emit_img(P):
            gg = gg_t[P]
            for b2 in range(2):
                img = img_pool.tile([W, W], f32, tag="img")
                img_t[2 * P + b2] = img
                base = SAMP_F * b2
                for c in range(CHUNKS):
                    nc.tensor.matmul(
                        img[:],
                        lhsT=gg[:, base + 2 * CW * c : base + 2 * CW * c + W],
                        rhs=gg[:, base + 2 * CW * c + CW : base + 2 * CW * c + 2 * CW],
                        start=(c == 0),
                        stop=(c == CHUNKS - 1),
                    )

        def emit_min_store(P):
            outp = out_pool.tile([W, 2 * W], f32, tag="op")
            outp_t[P] = outp
            for b2 in range(2):
                nc.vector.tensor_scalar(
                    outp[:, W * b2 : W * (b2 + 1)],
                    img_t[2 * P + b2][:],
                    DERF_FIX, 1.0,
                    mybir.AluOpType.mult, mybir.AluOpType.min,
                )
            nc.sync.dma_start(
                y_t.ap()[2 * P : 2 * P + 2].rearrange("b i j -> i b j"),
                outp[:].rearrange("i (b j) -> i b j", b=2),
            )

        # -------- software-pipelined emission --------
        emit_rmm(0)
        emit_rmm(1)
        emit_sub(0)
        for P in range(NPAIR):
            if P + 2 < NPAIR:
                emit_rmm(P + 2)
            if P + 1 < NPAIR:
                emit_sub(P + 1)
            emit_act(P)
            emit_img(P)
            if P >= 1:
                emit_min_store(P - 1)
        emit_min_store(NPAIR - 1)

    if legalize:
        _legalize_waits(nc)
    return nc


_PROGRAM = None


def kernel(x: np.ndarray, _trace: bool = False) -> np.ndarray:
    global _PROGRAM, LAST_RESULTS
    assert x.shape == (B, L, 8) and x.dtype == np.float32, (x.shape, x.dtype)
    if _PROGRAM is None:
        _PROGRAM = build_program()
    nc = _PROGRAM
    shards = np.split(np.ascontiguousarray(x), NCORES, axis=0)
    in_maps = [{"x": _host_ctrl(s)} for s in shards]
    res = run_bass_kernel_spmd(nc, in_maps, list(range(NCORES)), trace=_trace)
    LAST_RESULTS = res
    return np.concatenate([res.results[i]["y"] for i in range(NCORES)], axis=0)


# revision 6
# speedup vs baseline: 3.0368x; 3.0368x over previous
"""Bezier-to-image Gaussian splat kernel for Trainium2 (8 NeuronCores).

Reference computation (per sample b of 256):
    T = warped cubic Bernstein basis (30, 4)
    points = einsum('nk,blkc->blnc', T, x.reshape(B,160,4,2))   # (B,160,30,2)
    gx[b,l,i,n] = exp(-(i/60 - X[b,l,n])^2 / 2e-4)
    out[b,i,j]  = min(sum_{l,n} gx[b,l,i,n]*gy[b,l,j,n], 1)     # (B,60,60)

Strategy: pure data parallel, 32 samples per core.  The host pre-transposes
control points into a [20, 2560] layout (4 curve-strips x (4 ctrl rows +
ones row)) so the whole input is ONE contiguous DMA, and a single
block-diagonal [20,128] stationary computes r256 = round(256*60*X) for a
PAIR of samples per matmul.  The banded distance d256 = 256*i - r256 is an
all-int16 packed tensor_tensor (DVE 2x mode); the Gaussian is ONE
Derivative_Erf activation per pair (cost on ACT is free-size only), whose
output AP permutes the band into chunk-blocked layout for the 60x60 PSUM
accumulation matmuls.  Emission is software-pipelined (r two pairs ahead,
subtracts one pair ahead, clamps one pair behind) to keep ACT ~100% busy:
ACT is the roofline engine at ~8.4us/pair.
"""

import math

import numpy as np
import orjson

import bass_rust
import concourse.bass as bass
import concourse.mybir as mybir
import concourse.tile as tile
from concourse.bass_utils import run_bass_kernel_spmd

B, L, N, W = 256, 160, 30, 60
NCORES = 8
BC = B // NCORES          # samples per core (32)
NPAIR = BC // 2           # 16
ALPHA = 2e-4
KEXP = 1.0 / (W * W * ALPHA)          # exponent scale in cell units: 1/0.72
SDERF = math.sqrt(KEXP)               # Derivative_Erf input scale (per cell)
DERF_FIX = math.pi / 4.0              # undo (2/sqrt(pi))^2 from Derivative_Erf
CHUNKS = 40                           # 4 curves x 30 samples per chunk
PTS = 128                             # chunk partition dim: p = 32*lg + n
CW = 60                               # width of one chunk's band (= W)
R_HOLE = -15360.0                     # r256 for dead rows -> d256 large -> g=0
Q = 256.0                             # fixed-point scale (1/256 cell)

# If the hardware rejects int16 activation input, set to True: inserts a
# 4x-mode tensor_scalar converting d256 -> f16 cells before the LUT.
ACT_F16_FALLBACK = False

LAST_RESULTS = None  # test harness reads profiling info from here


def _basis_T() -> np.ndarray:
    t = np.arange(N, dtype=np.float32) / np.float32(N)
    t = 2 * t**3 - 3 * t**2 + 2 * t
    t_3_0 = t**3
    t_2_1 = t**2 - t_3_0
    t_1_2 = t_3_0 - 2 * t**2 + t
    t_0_3 = (1 - t) ** 3
    return np.stack([t_3_0, 3 * t_2_1, 3 * t_1_2, t_0_3], axis=1).astype(np.float32)


def _legalize_waits(nc, max_waits: int = 1):
    """Walrus rejects engine instructions carrying more than ~1 sync wait
    ("Too many sync wait commands").  Hoist excess waits onto same-engine
    Drain instructions inserted immediately before the offender."""
    js = orjson.loads(mybir.module_to_json_bytes(nc.m))
    ctr = 0
    for f in js["functions"]:
        for bb in f["blocks"]:
            out = []
            changed = False
            for inst in bb["instructions"]:
                si = inst.get("sync_info")
                waits = si.get("on_wait") if si else None
                if waits and len(waits) > max_waits:
                    keep = waits[:max_waits]
                    for w in waits[max_waits:]:
                        ctr += 1
                        out.append({
                            "debug": inst.get("debug", 0),
                            "engine": inst["engine"],
                            "ins": [], "outs": [],
                            "name": f"waitfix-{ctr}",
                            "opcode": "Drain",
                            "sync_info": {"on_update": [], "on_wait": [w]},
                        })
                    si["on_wait"] = keep
                    changed = True
                out.append(inst)
            if changed:
                bb["instructions"] = out
    if ctr:
        nc.m = bass_rust.module_from_json_bytes(orjson.dumps(js))
    return ctr


def _host_ctrl(x_core: np.ndarray) -> np.ndarray:
    """[32,160,8] f32 -> [20, 2560] f32: row 5*lg+k col (b,c,cc) =
    x[b, 4c+lg, 2k+cc] for k<4; row 5*lg+4 = 1.0 (ones row for holes)."""
    xr = x_core.reshape(BC, CHUNKS, 4, 4, 2)          # b, c, lg, k, cc
    arr = np.ones((4, 5, BC, CHUNKS, 2), dtype=np.float32)
    arr[:, :4] = xr.transpose(2, 3, 0, 1, 4)          # lg, k, b, c, cc
    return np.ascontiguousarray(arr.reshape(20, BC * CHUNKS * 2))


def build_program(legalize: bool = True):
    f32 = mybir.dt.float32
    f16 = mybir.dt.float16
    i16 = mybir.dt.int16

    nc = bass.Bass("TRN2", target_bir_lowering=False, debug=False)

    x_t = nc.dram_tensor("x", [20, BC * CHUNKS * 2], f32, kind="ExternalInput")
    y_t = nc.dram_tensor("y", [BC, W, W], f32, kind="ExternalOutput")

    # Block-diagonal stationary: col m = 32*lg + n gets 256*60*T[n,k] from
    # row 5*lg+k; hole cols n in {30,31} get R_HOLE via the ones row 5*lg+4.
    tsc_np = np.zeros((20, 128), dtype=np.float32)
    Tb = (Q * W) * _basis_T()                         # (30, 4)
    for lg in range(4):
        tsc_np[5 * lg : 5 * lg + 4, 32 * lg : 32 * lg + 30] = Tb.T
        tsc_np[5 * lg + 4, 32 * lg + 30 : 32 * lg + 32] = R_HOLE
    tsc_d = nc.inline_tensor(tsc_np, name="tscT")

    # iota: value 256*w at offset (w, s): [128, 120] int16
    iota_np = np.tile(
        np.repeat((Q * np.arange(CW)).astype(np.int16), 2)[None, :], (PTS, 1)
    )
    iota_d = nc.inline_tensor(iota_np, name="iota256")

    PAIR_F = 2 * CHUNKS * CW * 2                      # 9600 band elems per pair
    SAMP_F = 2 * CHUNKS * CW                          # 4800 per sample
    NSLICE = 8                                        # input DMA slices
    SCOL = (BC * CHUNKS * 2) // NSLICE                # 320 cols per slice

    with tile.TileContext(nc) as tc, tc.tile_pool(name="const", bufs=1) as cpool, \
            tc.tile_pool(name="ctrl", bufs=1) as ctrl_pool, \
            tc.tile_pool(name="outp", bufs=2) as out_pool, \
            tc.tile_pool(name="dd", bufs=2) as dd_pool, \
            tc.tile_pool(name="gg", bufs=2) as gg_pool, \
            tc.tile_pool(name="rps", bufs=2, space="PSUM") as rps_pool, \
            tc.tile_pool(name="img", bufs=4, space="PSUM") as img_pool:

        tsc = cpool.tile([20, 128], f32, tag="tsc")
        nc.sync.dma_start(tsc[:], tsc_d.ap())
        iot = cpool.tile([PTS, 2 * CW], i16, tag="iota")
        nc.sync.dma_start(iot[:], iota_d.ap())

        # control points: 8 independent column-slice tiles so each pair's
        # matmul only waits on its own slice's DMA.
        cts = []
        for s in range(NSLICE):
            ct_s = ctrl_pool.tile([20, SCOL], f32, tag=f"ct{s}")
            nc.sync.dma_start(ct_s[:], x_t.ap()[:, s * SCOL : (s + 1) * SCOL])
            cts.append(ct_s)

        # r256 for all 16 pairs lives in one persistent tile
        r_all = ctrl_pool.tile([PTS, NPAIR * 160], i16, tag="rall")

        dd_t = [None] * NPAIR
        gg_t = [None] * NPAIR
        img_t = [None] * BC
        outp_t = [None] * NPAIR

        def emit_rmm(P):
            sl = cts[P // 2]
            off = (P % 2) * 160
            r_ps = rps_pool.tile([PTS, 160], f32, tag="rps")
            nc.tensor.matmul(
                r_ps[:], lhsT=tsc[:], rhs=sl[:, off : off + 160],
                start=True, stop=True,
            )
            nc.vector.tensor_copy(r_all[:, 160 * P : 160 * P + 160], r_ps[:])

        def emit_sub(P):
            dd = dd_pool.tile([PTS, PAIR_F], i16, tag="dd")
            dd_t[P] = dd
            for b2 in range(2):
                # d256[p, (w, c, s)] = 256*w - r256[p, (c, s)]
                nc.vector.tensor_tensor(
                    dd[:, SAMP_F * b2 : SAMP_F * (b2 + 1)].rearrange(
                        "p (w c s) -> p w c s", w=CW, c=CHUNKS, s=2
                    ),
                    iot[:].rearrange("p (w o s) -> p w o s", o=1, s=2)
                    .broadcast_to([PTS, CW, CHUNKS, 2]),
                    r_all[:, 160 * P + 80 * b2 : 160 * P + 80 * (b2 + 1)]
                    .rearrange("p (o c s) -> p o c s", o=1, s=2)
                    .broadcast_to([PTS, CW, CHUNKS, 2]),
                    mybir.AluOpType.subtract,
                )

        def emit_act(P):
            # gg keeps dd's flat (b, w, cs) layout: ACT reads AND writes
            # fully packed 1-D (a permuted/strided ACT output AP measured
            # 5.3x slower on HW); the image matmuls take strided views.
            gg = gg_pool.tile([PTS, PAIR_F], f16, tag="gg")
            gg_t[P] = gg
            dd = dd_t[P]
            if ACT_F16_FALLBACK:
                df = dd_pool.tile([PTS, PAIR_F], f16, tag="df")
                nc.vector.tensor_scalar(
                    df[:], dd[:], 1.0 / Q, 0.0,
                    mybir.AluOpType.mult, mybir.AluOpType.add,
                )
                nc.scalar.activation(
                    gg[:], df[:], mybir.ActivationFunctionType.Derivative_Erf,
                    bias=0.0, scale=SDERF,
                )
            else:
                nc.scalar.activation(
                    gg[:], dd[:], mybir.ActivationFunctionType.Derivative_Erf,
                    bias=0.0, scale=SDERF / Q,
                )

        def emit_img(P):
            gg = gg_t[P]
            for b2 in range(2):
                img = img_pool.tile([W, W], f32, tag="img")
                img_t[2 * P + b2] = img
                # band of sample b2: element (w, cs) at b2*4800 + w*80 + cs
                gv = gg[:, SAMP_F * b2 : SAMP_F * (b2 + 1)].rearrange(
                    "p (w cs) -> p cs w", w=CW
                )
                for c in range(CHUNKS):
                    nc.tensor.matmul(
                        img[:],
                        lhsT=gv[:, 2 * c],
                        rhs=gv[:, 2 * c + 1],
                        start=(c == 0),
                        stop=(c == CHUNKS - 1),
                    )

        def emit_min_store(P):
            outp = out_pool.tile([W, 2 * W], f32, tag="op")
            outp_t[P] = outp
            for b2 in range(2):
                nc.vector.tensor_scalar(
                    outp[:, W * b2 : W * (b2 + 1)],
                    img_t[2 * P + b2][:],
                    DERF_FIX, 1.0,
                    mybir.AluOpType.mult, mybir.AluOpType.min,
                )
            nc.sync.dma_start(
                y_t.ap()[2 * P : 2 * P + 2].rearrange("b i j -> i b j"),
                outp[:].rearrange("i (b j) -> i b j", b=2),
            )

        # -------- software-pipelined emission --------
        emit_rmm(0)
        emit_rmm(1)
        emit_sub(0)
        for P in range(NPAIR):
            if P + 2 < NPAIR:
                emit_rmm(P + 2)
            if P + 1 < NPAIR:
                emit_sub(P + 1)
            emit_act(P)
            emit_img(P)
            if P >= 1:
                emit_min_store(P - 1)
        emit_min_store(NPAIR - 1)

    if legalize:
        _legalize_waits(nc)
    return nc


_PROGRAM = None


def kernel(x: np.ndarray, _trace: bool = False) -> np.ndarray:
    global _PROGRAM, LAST_RESULTS
    assert x.shape == (B, L, 8) and x.dtype == np.float32, (x.shape, x.dtype)
    if _PROGRAM is None:
        _PROGRAM = build_program()
    nc = _PROGRAM
    shards = np.split(np.ascontiguousarray(x), NCORES, axis=0)
    in_maps = [{"x": _host_ctrl(s)} for s in shards]
    res = run_bass_kernel_spmd(nc, in_maps, list(range(NCORES)), trace=_trace)
    LAST_RESULTS = res
    return np.concatenate([res.results[i]["y"] for i in range(NCORES)], axis=0)


# revision 13
# speedup vs baseline: 4.3439x; 1.4304x over previous
"""Bezier-to-image Gaussian splat kernel for Trainium2 (8 NeuronCores).

Reference computation (per sample b of 256):
    T = warped cubic Bernstein basis (30, 4)
    points = einsum('nk,blkc->blnc', T, x.reshape(B,160,4,2))   # (B,160,30,2)
    gx[b,l,i,n] = exp(-(i/60 - X[b,l,n])^2 / 2e-4)
    out[b,i,j]  = min(sum_{l,n} gx[b,l,i,n]*gy[b,l,j,n], 1)     # (B,60,60)

Strategy: pure data parallel, 32 samples per core.  The host pre-transposes
control points into a [20, 2560] layout (4 curve-strips x (4 ctrl rows +
ones row)) so the whole input is ONE contiguous DMA, and a single
block-diagonal [20,128] stationary computes r256 = round(256*60*X) for a
PAIR of samples per matmul.  The banded distance d256 = 256*i - r256 is an
all-int16 packed tensor_tensor (DVE 2x mode); the Gaussian is ONE
Derivative_Erf activation per pair (cost on ACT is free-size only), whose
output AP permutes the band into chunk-blocked layout for the 60x60 PSUM
accumulation matmuls.  Emission is software-pipelined (r two pairs ahead,
subtracts one pair ahead, clamps one pair behind) to keep ACT ~100% busy:
ACT is the roofline engine at ~8.4us/pair.
"""

import math

import numpy as np
import orjson

import bass_rust
import concourse.bass as bass
import concourse.mybir as mybir
import concourse.tile as tile
from concourse.bass_utils import run_bass_kernel_spmd

B, L, N, W = 256, 160, 30, 60
NCORES = 8
BC = B // NCORES          # samples per core (32)
NPAIR = BC // 2           # 16
ALPHA = 2e-4
KEXP = 1.0 / (W * W * ALPHA)          # exponent scale in cell units: 1/0.72
SDERF = math.sqrt(KEXP)               # Derivative_Erf input scale (per cell)
DERF_FIX = math.pi / 4.0              # undo (2/sqrt(pi))^2 from Derivative_Erf
CHUNKS = 40                           # 4 curves x 30 samples per chunk
PTS = 128                             # chunk partition dim: p = 32*lg + n
CW = 60                               # width of one chunk's band (= W)
R_HOLE = -15360.0                     # r256 for dead rows -> d256 large -> g=0
Q = 256.0                             # fixed-point scale (1/256 cell)

# If the hardware rejects int16 activation input, set to True: inserts a
# 4x-mode tensor_scalar converting d256 -> f16 cells before the LUT.
ACT_F16_FALLBACK = False

LAST_RESULTS = None  # test harness reads profiling info from here


def _basis_T() -> np.ndarray:
    t = np.arange(N, dtype=np.float32) / np.float32(N)
    t = 2 * t**3 - 3 * t**2 + 2 * t
    t_3_0 = t**3
    t_2_1 = t**2 - t_3_0
    t_1_2 = t_3_0 - 2 * t**2 + t
    t_0_3 = (1 - t) ** 3
    return np.stack([t_3_0, 3 * t_2_1, 3 * t_1_2, t_0_3], axis=1).astype(np.float32)


def _legalize_waits(nc, max_waits: int = 1):
    """Walrus rejects engine instructions carrying more than ~1 sync wait
    ("Too many sync wait commands").  Hoist excess waits onto same-engine
    Drain instructions inserted immediately before the offender."""
    js = orjson.loads(mybir.module_to_json_bytes(nc.m))
    ctr = 0
    for f in js["functions"]:
        for bb in f["blocks"]:
            out = []
            changed = False
            for inst in bb["instructions"]:
                si = inst.get("sync_info")
                waits = si.get("on_wait") if si else None
                if waits and len(waits) > max_waits:
                    keep = waits[:max_waits]
                    for w in waits[max_waits:]:
                        ctr += 1
                        out.append({
                            "debug": inst.get("debug", 0),
                            "engine": inst["engine"],
                            "ins": [], "outs": [],
                            "name": f"waitfix-{ctr}",
                            "opcode": "Drain",
                            "sync_info": {"on_update": [], "on_wait": [w]},
                        })
                    si["on_wait"] = keep
                    changed = True
                out.append(inst)
            if changed:
                bb["instructions"] = out
    if ctr:
        nc.m = bass_rust.module_from_json_bytes(orjson.dumps(js))
    return ctr


def _host_ctrl(x_core: np.ndarray) -> np.ndarray:
    """[32,160,8] f32 -> [20, 2560] f32: row 5*lg+k col (b,c,cc) =
    x[b, 4c+lg, 2k+cc] for k<4; row 5*lg+4 = 1.0 (ones row for holes)."""
    xr = x_core.reshape(BC, CHUNKS, 4, 4, 2)          # b, c, lg, k, cc
    arr = np.ones((4, 5, BC, CHUNKS, 2), dtype=np.float32)
    arr[:, :4] = xr.transpose(2, 3, 0, 1, 4)          # lg, k, b, c, cc
    return np.ascontiguousarray(arr.reshape(20, BC * CHUNKS * 2))


def build_program(legalize: bool = True):
    f32 = mybir.dt.float32
    f16 = mybir.dt.float16
    i16 = mybir.dt.int16

    nc = bass.Bass("TRN2", target_bir_lowering=False, debug=False)

    x_t = nc.dram_tensor("x", [20, BC * CHUNKS * 2], f32, kind="ExternalInput")
    y_t = nc.dram_tensor("y", [BC, W, W], f32, kind="ExternalOutput")

    # Block-diagonal stationary: col m = 32*lg + n gets 256*60*T[n,k] from
    # row 5*lg+k; hole cols n in {30,31} get R_HOLE via the ones row 5*lg+4.
    tsc_np = np.zeros((20, 128), dtype=np.float32)
    Tb = (Q * W) * _basis_T()                         # (30, 4)
    for lg in range(4):
        tsc_np[5 * lg : 5 * lg + 4, 32 * lg : 32 * lg + 30] = Tb.T
        tsc_np[5 * lg + 4, 32 * lg + 30 : 32 * lg + 32] = R_HOLE
    tsc_d = nc.inline_tensor(tsc_np, name="tscT")

    # iota: value 256*w at offset w: [128, 60] int16
    iota_np = np.tile((Q * np.arange(CW)).astype(np.int16)[None, :], (PTS, 1))
    iota_d = nc.inline_tensor(iota_np, name="iota256")

    PAIR_F = 2 * CHUNKS * CW * 2                      # 9600 band elems per pair
    SAMP_F = 2 * CHUNKS * CW                          # 4800 per sample
    NSLICE = 8                                        # input DMA slices
    SCOL = (BC * CHUNKS * 2) // NSLICE                # 320 cols per slice

    with tile.TileContext(nc) as tc, tc.tile_pool(name="const", bufs=1) as cpool, \
            tc.tile_pool(name="ctrl", bufs=1) as ctrl_pool, \
            tc.tile_pool(name="outp", bufs=2) as out_pool, \
            tc.tile_pool(name="dd", bufs=2) as dd_pool, \
            tc.tile_pool(name="gg", bufs=2) as gg_pool, \
            tc.tile_pool(name="rps", bufs=2, space="PSUM") as rps_pool, \
            tc.tile_pool(name="img", bufs=4, space="PSUM") as img_pool:

        tsc = cpool.tile([20, 128], f32, tag="tsc")
        nc.sync.dma_start(tsc[:], tsc_d.ap())
        iot = cpool.tile([PTS, CW], i16, tag="iota")
        nc.sync.dma_start(iot[:], iota_d.ap())

        # control points: 8 independent column-slice tiles so each pair's
        # matmul only waits on its own slice's DMA.
        cts = []
        for s in range(NSLICE):
            ct_s = ctrl_pool.tile([20, SCOL], f32, tag=f"ct{s}")
            nc.sync.dma_start(ct_s[:], x_t.ap()[:, s * SCOL : (s + 1) * SCOL])
            cts.append(ct_s)

        # r256 for all 16 pairs, each value duplicated x2 so the banded
        # subtract can keep a packed (stride-1, count-2) innermost dim on
        # the r operand while its output walks the chunk-blocked band
        # contiguously (DVE 2x mode needs every operand packed innermost).
        r_all = ctrl_pool.tile([PTS, NPAIR * 320], i16, tag="rall")

        dd_t = [None] * NPAIR
        gg_t = [None] * NPAIR
        img_t = [None] * BC
        outp_t = [None] * NPAIR

        def emit_rmm(P):
            sl = cts[P // 2]
            off = (P % 2) * 160
            r_ps = rps_pool.tile([PTS, 160], f32, tag="rps")
            nc.tensor.matmul(
                r_ps[:], lhsT=tsc[:], rhs=sl[:, off : off + 160],
                start=True, stop=True,
            )
            nc.vector.tensor_copy(
                r_all[:, 320 * P : 320 * P + 320].rearrange(
                    "p (cs d) -> p cs d", d=2
                ),
                r_ps[:].rearrange("p (cs o) -> p cs o", o=1)
                .broadcast_to([PTS, 160, 2]),
            )

        def emit_sub(P):
            dd = dd_pool.tile([PTS, PAIR_F], i16, tag="dd")
            dd_t[P] = dd
            for b2 in range(2):
                # d256[p, (cs, w)] = 256*w - r256[p, cs], chunk-blocked out.
                # Iteration (cs, w_hi, w_lo=2): out/iota walk contiguously,
                # r reads its duplicated pair -> all operands packed -> 2x.
                nc.vector.tensor_tensor(
                    dd[:, SAMP_F * b2 : SAMP_F * (b2 + 1)].rearrange(
                        "p (cs wh wl) -> p cs wh wl", cs=2 * CHUNKS, wl=2
                    ),
                    iot[:].rearrange("p (o wh wl) -> p o wh wl", o=1, wl=2)
                    .broadcast_to([PTS, 2 * CHUNKS, CW // 2, 2]),
                    r_all[:, 320 * P + 160 * b2 : 320 * P + 160 * (b2 + 1)]
                    .rearrange("p (cs o wl) -> p cs o wl", o=1, wl=2)
                    .broadcast_to([PTS, 2 * CHUNKS, CW // 2, 2]),
                    mybir.AluOpType.subtract,
                )

        def emit_act(P):
            # gg keeps dd's flat (b, w, cs) layout: ACT reads AND writes
            # fully packed 1-D (a permuted/strided ACT output AP measured
            # 5.3x slower on HW); the image matmuls take strided views.
            gg = gg_pool.tile([PTS, PAIR_F], f16, tag="gg")
            gg_t[P] = gg
            dd = dd_t[P]
            if ACT_F16_FALLBACK:
                df = dd_pool.tile([PTS, PAIR_F], f16, tag="df")
                nc.vector.tensor_scalar(
                    df[:], dd[:], 1.0 / Q, 0.0,
                    mybir.AluOpType.mult, mybir.AluOpType.add,
                )
                nc.scalar.activation(
                    gg[:], df[:], mybir.ActivationFunctionType.Derivative_Erf,
                    bias=0.0, scale=SDERF,
                )
            else:
                nc.scalar.activation(
                    gg[:], dd[:], mybir.ActivationFunctionType.Derivative_Erf,
                    bias=0.0, scale=SDERF / Q,
                )

        def emit_img(P):
            gg = gg_t[P]
            for b2 in range(2):
                img = img_pool.tile([W, W], f32, tag="img")
                img_t[2 * P + b2] = img
                base = SAMP_F * b2
                for c in range(CHUNKS):
                    nc.tensor.matmul(
                        img[:],
                        lhsT=gg[:, base + 2 * CW * c : base + 2 * CW * c + W],
                        rhs=gg[:, base + 2 * CW * c + CW : base + 2 * CW * c + 2 * CW],
                        start=(c == 0),
                        stop=(c == CHUNKS - 1),
                    )

        def emit_min_store(P):
            outp = out_pool.tile([W, 2 * W], f32, tag="op")
            outp_t[P] = outp
            for b2 in range(2):
                nc.vector.tensor_scalar(
                    outp[:, W * b2 : W * (b2 + 1)],
                    img_t[2 * P + b2][:],
                    DERF_FIX, 1.0,
                    mybir.AluOpType.mult, mybir.AluOpType.min,
                )
            nc.sync.dma_start(
                y_t.ap()[2 * P : 2 * P + 2].rearrange("b i j -> i b j"),
                outp[:].rearrange("i (b j) -> i b j", b=2),
            )

        # -------- software-pipelined emission --------
        emit_rmm(0)
        emit_rmm(1)
        emit_sub(0)
        for P in range(NPAIR):
            if P + 2 < NPAIR:
                emit_rmm(P + 2)
            if P + 1 < NPAIR:
                emit_sub(P + 1)
            emit_act(P)
            emit_img(P)
            if P >= 1:
                emit_min_store(P - 1)
        emit_min_store(NPAIR - 1)

    if legalize:
        _legalize_waits(nc)
    return nc


_PROGRAM = None


def kernel(x: np.ndarray, _trace: bool = False) -> np.ndarray:
    global _PROGRAM, LAST_RESULTS
    assert x.shape == (B, L, 8) and x.dtype == np.float32, (x.shape, x.dtype)
    if _PROGRAM is None:
        _PROGRAM = build_program()
    nc = _PROGRAM
    shards = np.split(np.ascontiguousarray(x), NCORES, axis=0)
    in_maps = [{"x": _host_ctrl(s)} for s in shards]
    res = run_bass_kernel_spmd(nc, in_maps, list(range(NCORES)), trace=_trace)
    LAST_RESULTS = res
    return np.concatenate([res.results[i]["y"] for i in range(NCORES)], axis=0)
